# revision 7
# baseline (speedup 1.0000x reference)
"""ContrastLoss kernel for Trainium2 (8 NeuronCores, SPMD data-parallel).

loss = -sum_i dot(f_s[i], f_t[i]) / B  ==  -sum(f_s * f_t) / B

The row structure is irrelevant: the answer is the global sum of the
elementwise product. Each core gets 1/8 of the batch (a flat 4M-element
chunk viewed as [128, 32768]), computes per-partition partial sums with
fused DVE tensor_tensor_reduce ops, and the host sums the 8x[128 x T]
partials and applies -1/B.
"""

import sys

for _p in (
    "/root/.axon_site",
    "/root/.axon_site/_ro/trn_rl_repo",
    "/root/.axon_site/_ro/pypackages",
    "/opt/trn_rl_repo",
    "/opt/pypackages",
):
    if _p not in sys.path:
        sys.path.append(_p)

import numpy as np

B, D = 65536, 512
N_CORES = 8
P = 128
ROWS_PER_CORE = B // N_CORES              # 8192
FREE = ROWS_PER_CORE * D // P             # 32768 f32 per partition per tensor
TILE_N = 4096                             # 128 x 4096 f32 = 2 MiB per DMA
N_TILES = FREE // TILE_N                  # 8

_CACHE = {}


def _build():
    from contextlib import ExitStack

    import concourse.bacc as bacc
    import concourse.mybir as mybir
    import concourse.tile as tile

    nc = bacc.Bacc(
        "TRN2", target_bir_lowering=False, debug=False, num_devices=N_CORES
    )
    a = nc.declare_dram_parameter("a", [P, FREE], mybir.dt.float32, isOutput=False)
    b = nc.declare_dram_parameter("b", [P, FREE], mybir.dt.float32, isOutput=False)
    out = nc.declare_dram_parameter(
        "partials", [P, N_TILES], mybir.dt.float32, isOutput=True
    )

    with tile.TileContext(nc) as tc, ExitStack() as ctx:
        pa = ctx.enter_context(tc.tile_pool(name="pa", bufs=3))
        pb = ctx.enter_context(tc.tile_pool(name="pb", bufs=3))
        pm = ctx.enter_context(tc.tile_pool(name="pm", bufs=2))
        pacc = ctx.enter_context(tc.tile_pool(name="pacc", bufs=1))

        acc = pacc.tile([P, N_TILES], mybir.dt.float32)
        for t in range(N_TILES):
            sl = slice(t * TILE_N, (t + 1) * TILE_N)
            ta = pa.tile([P, TILE_N], mybir.dt.float32)
            nc.sync.dma_start(out=ta[:], in_=a[:, sl])
            tb = pb.tile([P, TILE_N], mybir.dt.float32)
            nc.sync.dma_start(out=tb[:], in_=b[:, sl])
            tm = pm.tile([P, TILE_N], mybir.dt.float32)
            nc.vector.tensor_mul(tm[:], ta[:], tb[:])
            tj = pm.tile([P, TILE_N], mybir.dt.float32, tag="junk")
            nc.scalar.activation(
                out=tj[:],
                in_=tm[:],
                func=mybir.ActivationFunctionType.Copy,
                accum_out=acc[:, t : t + 1],
            )
        nc.sync.dma_start(out=out[:], in_=acc[:])
    nc.compile()
    return nc


def _get_nc():
    if "nc" not in _CACHE:
        _CACHE["nc"] = _build()
    return _CACHE["nc"]


def run(f_s, f_t, trace=False):
    """Returns (loss ndarray shape (1,) f32, exec_time_ns or None)."""
    from concourse.bass_utils import run_bass_kernel_spmd

    f_s = np.ascontiguousarray(np.asarray(f_s, dtype=np.float32))
    f_t = np.ascontiguousarray(np.asarray(f_t, dtype=np.float32))
    assert f_s.shape == (B, D) and f_t.shape == (B, D)

    in_maps = []
    for c in range(N_CORES):
        rows = slice(c * ROWS_PER_CORE, (c + 1) * ROWS_PER_CORE)
        in_maps.append(
            {
                "a": f_s[rows].reshape(P, FREE),
                "b": f_t[rows].reshape(P, FREE),
            }
        )

    res = run_bass_kernel_spmd(_get_nc(), in_maps, list(range(N_CORES)), trace=trace)
    total = np.float64(0.0)
    for r in res.results:
        total += r["partials"].astype(np.float64).sum()
    loss = np.asarray([-total / B], dtype=np.float32)
    return loss, res.exec_time_ns


def kernel(f_s, f_t):
    return run(f_s, f_t, trace=False)[0]


# revision 8
# speedup vs baseline: 1.0861x; 1.0861x over previous
"""ContrastLoss kernel for Trainium2 (8 NeuronCores, SPMD data-parallel).

loss = -sum_i dot(f_s[i], f_t[i]) / B  ==  -sum(f_s * f_t) / B

The row structure is irrelevant: the answer is the global sum of the
elementwise product. Each core gets 1/8 of the batch (a flat 4M-element
chunk viewed as [128, 32768]), computes per-partition partial sums with
fused DVE tensor_tensor_reduce ops, and the host sums the 8x[128 x T]
partials and applies -1/B.
"""

import sys

for _p in (
    "/root/.axon_site",
    "/root/.axon_site/_ro/trn_rl_repo",
    "/root/.axon_site/_ro/pypackages",
    "/opt/trn_rl_repo",
    "/opt/pypackages",
):
    if _p not in sys.path:
        sys.path.append(_p)

import numpy as np

B, D = 65536, 512
N_CORES = 8
P = 128
ROWS_PER_CORE = B // N_CORES              # 8192
FREE = ROWS_PER_CORE * D // P             # 32768 f32 per partition per tensor
TILE_N = 4096                             # 128 x 4096 f32 = 2 MiB per DMA
N_TILES = FREE // TILE_N                  # 8

_CACHE = {}


def _build():
    from contextlib import ExitStack

    import concourse.bacc as bacc
    import concourse.mybir as mybir
    import concourse.tile as tile

    nc = bacc.Bacc(
        "TRN2", target_bir_lowering=False, debug=False, num_devices=N_CORES
    )
    a = nc.declare_dram_parameter("a", [P, FREE], mybir.dt.float32, isOutput=False)
    b = nc.declare_dram_parameter("b", [P, FREE], mybir.dt.float32, isOutput=False)
    out = nc.declare_dram_parameter(
        "partials", [P, N_TILES], mybir.dt.float32, isOutput=True
    )

    with tile.TileContext(nc) as tc, ExitStack() as ctx:
        pa = ctx.enter_context(tc.tile_pool(name="pa", bufs=3))
        pb = ctx.enter_context(tc.tile_pool(name="pb", bufs=3))
        pm = ctx.enter_context(tc.tile_pool(name="pm", bufs=2))
        pacc = ctx.enter_context(tc.tile_pool(name="pacc", bufs=1))

        acc = pacc.tile([P, N_TILES], mybir.dt.float32)
        for t in range(N_TILES):
            sl = slice(t * TILE_N, (t + 1) * TILE_N)
            ta = pa.tile([P, TILE_N], mybir.dt.float32)
            nc.sync.dma_start(out=ta[:], in_=a[:, sl])
            tb = pb.tile([P, TILE_N], mybir.dt.float32)
            nc.sync.dma_start(out=tb[:], in_=b[:, sl])
            tm = pm.tile([P, TILE_N], mybir.dt.float32)
            nc.vector.tensor_mul(tm[:], ta[:], tb[:])
            tj = pm.tile([P, TILE_N], mybir.dt.float32, tag="junk")
            nc.scalar.activation(
                out=tj[:],
                in_=tm[:],
                func=mybir.ActivationFunctionType.Copy,
                accum_out=acc[:, t : t + 1],
            )
        nc.sync.dma_start(out=out[:], in_=acc[:])
    nc.compile()
    return nc


def _get_nc():
    if "nc" not in _CACHE:
        _CACHE["nc"] = _build()
    return _CACHE["nc"]


def run(f_s, f_t, trace=False):
    """Returns (loss ndarray shape (1,) f32, exec_time_ns or None)."""
    from concourse.bass_utils import run_bass_kernel_spmd

    f_s = np.ascontiguousarray(np.asarray(f_s, dtype=np.float32))
    f_t = np.ascontiguousarray(np.asarray(f_t, dtype=np.float32))
    assert f_s.shape == (B, D) and f_t.shape == (B, D)

    in_maps = []
    for c in range(N_CORES):
        rows = slice(c * ROWS_PER_CORE, (c + 1) * ROWS_PER_CORE)
        in_maps.append(
            {
                "a": f_s[rows].reshape(P, FREE),
                "b": f_t[rows].reshape(P, FREE),
            }
        )

    res = run_bass_kernel_spmd(_get_nc(), in_maps, list(range(N_CORES)), trace=trace)
    _CACHE["last_results"] = res
    total = np.float64(0.0)
    for r in res.results:
        total += r["partials"].astype(np.float64).sum()
    loss = np.asarray([-total / B], dtype=np.float32)
    return loss, res.exec_time_ns


def kernel(f_s, f_t):
    return run(f_s, f_t, trace=False)[0]
